# revision 2
# baseline (speedup 1.0000x reference)
"""Trainium2 Bass kernel for a 4-layer dense transformer (B=1, S=2048, D=1024,
H=16, DK=64, FF=4096, V=50000) distributed over 8 NeuronCores.

Sharding:
  - Attention: tensor-parallel over heads (2 heads/core), full sequence.
  - LayerNorm / FFN / residual: sequence-parallel (256 rows/core), full width.
  - Vocab projection: sharded over vocab (6250 cols/core).
  - Glue per layer: AllGather of x^T (for QKV inputs) and AllToAll of the
    normalized ctx^T (delivers every head's dims for the core's own rows).
    One final AllGather before the vocab matmul.

Matmuls run in float32r (TF32) with fp32 PSUM accumulation. Softmax is
computed without max-subtraction (scores are O(1) here), with the causal mask
applied multiplicatively after exp, and the denominator obtained via an
appended ones-column in the PV matmul.
"""
import sys

if "/opt/trn_rl_repo" not in sys.path:
    sys.path.insert(0, "/opt/trn_rl_repo")

import contextlib

import numpy as np

import concourse.bass as bass
import concourse.tile as tile
from concourse import bacc, mybir
from concourse.bass_utils import run_bass_kernel_spmd
from concourse.masks import make_identity

F32 = mybir.dt.float32
F32R = mybir.dt.float32r
I32 = mybir.dt.int32
AF = mybir.ActivationFunctionType

NC = 8                    # cores
B, S, D, H, DK, FF, V, L = 1, 2048, 1024, 16, 64, 4096, 50000, 4
EPS = 1e-5
SCALE = 1.0 / np.sqrt(DK)
HL = H // NC              # heads per core = 2
DHL = HL * DK             # local head dims = 128
SL = S // NC              # rows per core = 256
VL = V // NC              # vocab per core = 6250
KC = D // 128             # contraction chunks over D = 8
VCH = [512] * 12 + [106]  # vocab free chunks (12*512+106 = 6250)

_CACHE = {}


def _np_rope_tables():
    inv_freq = 1.0 / (10000.0 ** (np.arange(0, DK, 2, dtype=np.float32) / DK))
    t = np.arange(S, dtype=np.float32)
    freqs = np.outer(t, inv_freq)                 # [S, DK/2]
    emb = np.concatenate([freqs, freqs], -1)      # [S, DK]
    return np.cos(emb), np.sin(emb)


def _diag_masks():
    # expT tile layout: [128 keys, 512 q]; for diag chunk d (0..3):
    # allowed iff q >= d*128 + k
    masks = np.zeros((4, 128, 512), np.float32)
    k = np.arange(128)[:, None]
    q = np.arange(512)[None, :]
    for d in range(4):
        masks[d] = (q >= d * 128 + k).astype(np.float32)
    return masks


def build_program(n_layers=L, debug_x=False):
    nc = bacc.Bacc("TRN2", target_bir_lowering=False, debug=False,
                   num_devices=NC)

    t = {}
    t["ids"] = nc.dram_tensor("ids", [2, 128, 1], I32, kind="ExternalInput")
    t["temb"] = nc.dram_tensor("token_emb", [V, D], F32, kind="ExternalInput")
    t["pemb"] = nc.dram_tensor("pos_emb", [SL, D], F32, kind="ExternalInput")
    t["wq"] = nc.dram_tensor("wq", [L, D, DHL], F32R, kind="ExternalInput")
    t["wk"] = nc.dram_tensor("wk", [L, D, DHL], F32R, kind="ExternalInput")
    t["wv"] = nc.dram_tensor("wv", [L, D, DHL], F32R, kind="ExternalInput")
    t["wo"] = nc.dram_tensor("wo_w", [L, D, D], F32R, kind="ExternalInput")
    t["wob"] = nc.dram_tensor("wo_b", [L, D], F32, kind="ExternalInput")
    t["ln1w"] = nc.dram_tensor("ln1_w", [L, D], F32, kind="ExternalInput")
    t["ln1b"] = nc.dram_tensor("ln1_b", [L, D], F32, kind="ExternalInput")
    t["ln2w"] = nc.dram_tensor("ln2_w", [L, D], F32, kind="ExternalInput")
    t["ln2b"] = nc.dram_tensor("ln2_b", [L, D], F32, kind="ExternalInput")
    t["ff1"] = nc.dram_tensor("ff1_w", [L, D, FF], F32R, kind="ExternalInput")
    t["ff1b"] = nc.dram_tensor("ff1_b", [L, FF], F32, kind="ExternalInput")
    t["ff2"] = nc.dram_tensor("ff2_w", [L, FF, D], F32R, kind="ExternalInput")
    t["ff2b"] = nc.dram_tensor("ff2_b", [L, D], F32, kind="ExternalInput")
    t["outw"] = nc.dram_tensor("out_w", [D, VL], F32R, kind="ExternalInput")
    t["outb"] = nc.dram_tensor("out_b", [VL], F32, kind="ExternalInput")
    t["cos"] = nc.dram_tensor("cosT", [128, S], F32, kind="ExternalInput")
    t["sin"] = nc.dram_tensor("sinT", [128, S], F32, kind="ExternalInput")
    t["dmask"] = nc.dram_tensor("dmask", [4, 128, 512], F32R,
                                kind="ExternalInput")

    t["logits"] = nc.dram_tensor("logits", [S, VL], F32, kind="ExternalOutput")
    if debug_x:
        t["dbg_x"] = nc.dram_tensor("dbg_x", [SL, D], F32,
                                    kind="ExternalOutput")

    # collective bounce buffers
    t["xt_in"] = [nc.dram_tensor(f"xt_in_{l}", [D, SL], F32R)
                  for l in range(n_layers + 1)]
    t["xt_out"] = [nc.dram_tensor(f"xt_out_{l}", [NC * D, SL], F32R,
                                  addr_space="Shared")
                   for l in range(n_layers + 1)]
    t["cx_in"] = [nc.dram_tensor(f"cx_in_{l}", [NC * 128, SL], F32R)
                  for l in range(n_layers)]
    t["cx_out"] = [nc.dram_tensor(f"cx_out_{l}", [NC * 128, SL], F32R)
                   for l in range(n_layers)]

    with tile.TileContext(nc) as tc:
        _build(nc, tc, t, n_layers, debug_x)
    nc.compile()
    return nc


def _build(nc, tc, t, n_layers, debug_x):
    rg = [list(range(NC))]
    es = contextlib.ExitStack()
    with es:
        const = es.enter_context(tc.tile_pool(name="const", bufs=1))
        glob = es.enter_context(tc.tile_pool(name="glob", bufs=1))
        gps = es.enter_context(tc.tile_pool(name="gps", bufs=2, space="PSUM"))

        # ---------------- constants ----------------
        ident = const.tile([128, 128], F32)
        make_identity(nc, ident[:])
        cos_t = const.tile([128, S], F32)
        sin_t = const.tile([128, S], F32)
        nc.sync.dma_start(cos_t[:], t["cos"][:, :])
        nc.sync.dma_start(sin_t[:], t["sin"][:, :])
        dmask_t = const.tile([128, 4, 512], F32R)
        for d in range(4):
            nc.sync.dma_start(dmask_t[:, d, :], t["dmask"][d, :, :])
        ones_t = const.tile([128, 16], F32)
        nc.vector.memset(ones_t[:], 1.0)

        def bcast_load(dst, src_1d):
            """DMA a [N] DRAM vector into a [P, N] tile, replicated."""
            p = dst.shape[0]
            ap = bass.AP(tensor=src_1d.tensor, offset=src_1d.offset,
                         ap=[[0, p]] + src_1d.ap)
            nc.sync.dma_start(dst, ap)

        # x_own[m]: [128, 1024] f32, own rows (m=0: rows 0..127 of the
        # core's 256; m=1: rows 128..255)
        x_own = [glob.tile([128, D], F32, name=f"x_own{m}") for m in range(2)]

        def transpose_to_xt(src_tiles, dst_dram, pool):
            """src [2][128, 1024] f32 -> dst_dram [1024, 256] f32r via PE."""
            xt_sb = pool.tile([128, KC, 256], F32R, name="xt_sb")
            for m in range(2):
                for kc in range(KC):
                    tp = gps.tile([128, 128], F32, name="tp_ps")
                    nc.tensor.transpose(
                        tp[:], src_tiles[m][:, kc * 128:(kc + 1) * 128],
                        ident[:])
                    nc.vector.tensor_copy(
                        xt_sb[:, kc, m * 128:(m + 1) * 128], tp[:])
            for kc in range(KC):
                nc.sync.dma_start(
                    dst_dram[kc * 128:(kc + 1) * 128, :], xt_sb[:, kc, :])

        def layer_norm(dst, src, w_t, b_t, small):
            st = small.tile([128, 2, 6], F32, name="bn_st")
            mv = small.tile([128, 2], F32, name="bn_mv")
            for g in range(2):
                nc.vector.bn_stats(st[:, g, :],
                                   src[:, g * 512:(g + 1) * 512])
            nc.vector.bn_aggr(mv[:], st[:])
            rstd = small.tile([128, 1], F32, name="rstd")
            eps_t = small.tile([128, 1], F32, name="eps")
            nc.vector.memset(eps_t[:], EPS)
            nc.scalar.activation(rstd[:], mv[:, 1:2], AF.Sqrt, bias=eps_t[:])
            nc.vector.reciprocal(rstd[:], rstd[:])
            nc.vector.tensor_scalar(
                out=dst[:], in0=src[:], scalar1=mv[:, 0:1], scalar2=rstd[:],
                op0=mybir.AluOpType.subtract, op1=mybir.AluOpType.mult)
            nc.vector.tensor_mul(dst[:], dst[:], w_t[:])
            nc.vector.tensor_add(dst[:], dst[:], b_t[:])

        # ---------------- embedding ----------------
        with tc.tile_pool(name="emb", bufs=2) as emb:
            for m in range(2):
                idx_t = emb.tile([128, 1], I32, name="idx")
                nc.sync.dma_start(idx_t[:], t["ids"][m, :, :])
                gat = emb.tile([128, D], F32, name="gat")
                nc.gpsimd.indirect_dma_start(
                    out=gat[:], out_offset=None, in_=t["temb"][:, :],
                    in_offset=bass.IndirectOffsetOnAxis(ap=idx_t[:, :1],
                                                        axis=0))
                pos_t = emb.tile([128, D], F32, name="pos")
                nc.sync.dma_start(pos_t[:],
                                  t["pemb"][m * 128:(m + 1) * 128, :])
                nc.vector.tensor_add(x_own[m][:], gat[:], pos_t[:])
            transpose_to_xt(x_own, t["xt_in"][0], emb)
        nc.gpsimd.collective_compute(
            "AllGather", mybir.AluOpType.bypass, replica_groups=rg,
            ins=[t["xt_in"][0][:, :]], outs=[t["xt_out"][0][:, :]])

        # ---------------- layers ----------------
        for l in range(n_layers):
            with tc.tile_pool(name=f"layer{l}", bufs=1) as lp:
                qT = lp.tile([128, S], F32, name="qT")
                kT = lp.tile([128, S], F32, name="kT")
                v_aug = [lp.tile([128, 16, 65], F32R, name=f"vaug{h}")
                         for h in range(HL)]
                ctxc = lp.tile([128, S], F32R, name="ctxc")
                xln = [lp.tile([128, D], F32, name=f"xln{m}")
                       for m in range(2)]
                z = [lp.tile([128, D], F32, name=f"zz{m}") for m in range(2)]

                # ---- phase 1: QKV ----
                with tc.tile_pool(name="ph_qkv", bufs=2) as pp, \
                     tc.tile_pool(name="ps_qkv", bufs=2, space="PSUM") as pq:
                    wq_sb = pp.tile([128, KC, DHL], F32R, name="wq_sb")
                    wk_sb = pp.tile([128, KC, DHL], F32R, name="wk_sb")
                    wv_sb = pp.tile([128, KC, DHL], F32R, name="wv_sb")
                    nc.sync.dma_start(
                        wq_sb[:],
                        t["wq"][l].rearrange("(kc p) m -> p kc m", p=128))
                    nc.sync.dma_start(
                        wk_sb[:],
                        t["wk"][l].rearrange("(kc p) m -> p kc m", p=128))
                    nc.sync.dma_start(
                        wv_sb[:],
                        t["wv"][l].rearrange("(kc p) m -> p kc m", p=128))
                    for h in range(HL):
                        nc.vector.tensor_copy(v_aug[h][:, :, 64], ones_t[:])
                    for rb in range(NC):
                        xt_c = pp.tile([128, KC, 256], F32R, name="xt_c")
                        nc.gpsimd.dma_start(
                            xt_c[:],
                            t["xt_out"][l][rb * D:(rb + 1) * D, :]
                            .rearrange("(kc p) s -> p kc s", p=128))
                        for (w_sb, dstT) in ((wq_sb, qT), (wk_sb, kT)):
                            pt = pq.tile([128, 256], F32, name="qk_ps")
                            for kc in range(KC):
                                nc.tensor.matmul(
                                    pt[:], w_sb[:, kc, :], xt_c[:, kc, :],
                                    start=(kc == 0), stop=(kc == KC - 1))
                            nc.vector.tensor_copy(
                                dstT[:, rb * 256:(rb + 1) * 256], pt[:])
                        for half in range(2):
                            sc = rb * 2 + half
                            pt = pq.tile([128, DHL], F32, name="v_ps")
                            for kc in range(KC):
                                nc.tensor.matmul(
                                    pt[:],
                                    xt_c[:, kc, half * 128:(half + 1) * 128],
                                    wv_sb[:, kc, :],
                                    start=(kc == 0), stop=(kc == KC - 1))
                            for h in range(HL):
                                nc.vector.tensor_copy(
                                    v_aug[h][:, sc, 0:64],
                                    pt[:, h * 64:(h + 1) * 64])

                # ---- phase 2: RoPE ----
                qTr_t = lp.tile([128, S], F32R, name="qTr")
                kTr_t = lp.tile([128, S], F32R, name="kTr")
                with tc.tile_pool(name="ph_rope", bufs=2) as pp:
                    for srcT, dstT in ((qT, qTr_t), (kT, kTr_t)):
                        rh = pp.tile([128, S], F32, name="rope_rh")
                        for h in range(HL):
                            lo, hi = h * 64, h * 64 + 32
                            nc.scalar.mul(rh[lo:hi, :], srcT[hi:hi + 32, :],
                                          -1.0)
                            nc.scalar.copy(rh[hi:hi + 32, :], srcT[lo:hi, :])
                        t2 = pp.tile([128, S], F32, name="rope_t2")
                        nc.vector.tensor_mul(t2[:], srcT[:], cos_t[:])
                        nc.vector.tensor_mul(rh[:], rh[:], sin_t[:])
                        nc.vector.tensor_add(dstT[:], t2[:], rh[:])
                qTr = qTr_t[:]
                kTr = kTr_t[:]

                # ---- phase 3: attention ----
                with tc.tile_pool(name="ph_att", bufs=2) as pp, \
                     tc.tile_pool(name="ps_sc", bufs=2, space="PSUM") as psc, \
                     tc.tile_pool(name="ps_ct", bufs=1, space="PSUM") as pct, \
                     tc.tile_pool(name="att_sm", bufs=3) as sm:
                    for qb in range(4):
                        nkc = (qb + 1) * 4
                        ct_ps = [pct.tile([65, 512], F32, name=f"ct_ps{h}")
                                 for h in range(HL)]
                        for kc in range(nkc):
                            e_sb = [None, None]
                            for h in range(HL):
                                sc_ps = psc.tile([128, 512], F32,
                                                 name=f"sc_ps{h}")
                                nc.tensor.matmul(
                                    sc_ps[:],
                                    kTr[h * 64:(h + 1) * 64,
                                        kc * 128:(kc + 1) * 128],
                                    qTr[h * 64:(h + 1) * 64,
                                        qb * 512:(qb + 1) * 512],
                                    start=True, stop=True,
                                    tile_position=(64 * h, 0))
                                et = pp.tile([128, 512], F32R,
                                             name=f"exp{h}")
                                nc.scalar.activation(et[:], sc_ps[:], AF.Exp,
                                                     scale=float(SCALE))
                                d = kc - qb * 4
                                if d >= 0:
                                    nc.vector.tensor_mul(
                                        et[:], et[:], dmask_t[:, d, :])
                                e_sb[h] = et
                            for h in range(HL):
                                nc.tensor.matmul(
                                    ct_ps[h][:], v_aug[h][:, kc, :],
                                    e_sb[h][:],
                                    start=(kc == 0), stop=(kc == nkc - 1))
                        for h in range(HL):
                            rec = sm.tile([1, 512], F32, name="rec")
                            nc.vector.tensor_copy(rec[:], ct_ps[h][64:65, :])
                            bc = sm.tile([64, 512], F32, name="bc")
                            nc.gpsimd.partition_broadcast(bc[:], rec[:])
                            nc.vector.reciprocal(bc[:], bc[:])
                            nc.vector.tensor_mul(
                                ctxc[h * 64:(h + 1) * 64,
                                     qb * 512:(qb + 1) * 512],
                                ct_ps[h][0:64, :], bc[:])

                # ---- AllToAll ctx: shard j = ctxc[:, j*256:(j+1)*256] ----
                for j in range(NC):
                    nc.sync.dma_start(
                        t["cx_in"][l][j * 128:(j + 1) * 128, :],
                        ctxc[:, j * SL:(j + 1) * SL])
                nc.gpsimd.collective_compute(
                    "AllToAll", mybir.AluOpType.bypass, replica_groups=rg,
                    ins=[t["cx_in"][l][:, :]], outs=[t["cx_out"][l][:, :]])

                # ---- phase 4: out-proj + LN1 ----
                with tc.tile_pool(name="ph_wo", bufs=1) as pp, \
                     tc.tile_pool(name="wo_sm", bufs=3) as sm, \
                     tc.tile_pool(name="ps_wo", bufs=2, space="PSUM") as pw:
                    ctxT = pp.tile([128, KC, 256], F32R, name="ctxT")
                    for rb in range(NC):
                        nc.gpsimd.dma_start(
                            ctxT[:, rb, :],
                            t["cx_out"][l][rb * 128:(rb + 1) * 128, :])
                    wo_sb = pp.tile([128, KC, D], F32R, name="wo_sb")
                    nc.sync.dma_start(
                        wo_sb[:],
                        t["wo"][l].rearrange("(kc p) n -> p kc n", p=128))
                    wob_t = sm.tile([128, D], F32, name="wob")
                    bcast_load(wob_t[:], t["wob"][l])
                    ln1w_t = sm.tile([128, D], F32, name="ln1w")
                    ln1b_t = sm.tile([128, D], F32, name="ln1b")
                    bcast_load(ln1w_t[:], t["ln1w"][l])
                    bcast_load(ln1b_t[:], t["ln1b"][l])
                    for m in range(2):
                        for n in range(2):
                            yp = pw.tile([128, 512], F32, name="y_ps")
                            for kc in range(KC):
                                nc.tensor.matmul(
                                    yp[:], ctxT[:, kc, m * 128:(m + 1) * 128],
                                    wo_sb[:, kc, n * 512:(n + 1) * 512],
                                    start=(kc == 0), stop=(kc == KC - 1))
                            nsl = slice(n * 512, (n + 1) * 512)
                            nc.vector.tensor_add(z[m][:, nsl], yp[:],
                                                 x_own[m][:, nsl])
                        nc.vector.tensor_add(z[m][:], z[m][:], wob_t[:])
                        layer_norm(xln[m], z[m], ln1w_t, ln1b_t, sm)

                # ---- phase 5: FFN + LN2 ----
                with tc.tile_pool(name="ph_ff", bufs=1) as pp, \
                     tc.tile_pool(name="ff_st", bufs=3) as st, \
                     tc.tile_pool(name="ff_sm", bufs=3) as sm, \
                     tc.tile_pool(name="ps_h", bufs=2, space="PSUM") as ph, \
                     tc.tile_pool(name="ps_y2", bufs=1, space="PSUM") as py2:
                    xlnT = pp.tile([128, KC, 256], F32R, name="xlnT")
                    for m in range(2):
                        for kc in range(KC):
                            tp = gps.tile([128, 128], F32, name="tp_ps")
                            nc.tensor.transpose(
                                tp[:], xln[m][:, kc * 128:(kc + 1) * 128],
                                ident[:])
                            nc.vector.tensor_copy(
                                xlnT[:, kc, m * 128:(m + 1) * 128], tp[:])
                    ff1b_t = sm.tile([128, FF // 128], F32, name="ff1b")
                    nc.sync.dma_start(
                        ff1b_t[:],
                        t["ff1b"][l].rearrange("(m p) -> p m", p=128))
                    hT = pp.tile([128, FF // 128, 256], F32R, name="hT")
                    ff1v = t["ff1"][l].rearrange("(kc p) m -> p kc m", p=128)
                    for mh in range(FF // 128):
                        f1t = st.tile([128, KC, 128], F32R, name="f1t")
                        nc.sync.dma_start(
                            f1t[:], ff1v[:, :, mh * 128:(mh + 1) * 128])
                        hp = ph.tile([128, 256], F32, name="h_ps")
                        for kc in range(KC):
                            nc.tensor.matmul(
                                hp[:], f1t[:, kc, :], xlnT[:, kc, :],
                                start=(kc == 0), stop=(kc == KC - 1))
                        nc.scalar.activation(hT[:, mh, :], hp[:], AF.Gelu,
                                             bias=ff1b_t[:, mh:mh + 1])

                    ff2b_t = sm.tile([128, D], F32, name="ff2b")
                    bcast_load(ff2b_t[:], t["ff2b"][l])
                    ln2w_t = sm.tile([128, D], F32, name="ln2w")
                    ln2b_t = sm.tile([128, D], F32, name="ln2b")
                    bcast_load(ln2w_t[:], t["ln2w"][l])
                    bcast_load(ln2b_t[:], t["ln2b"][l])
                    ff2v = t["ff2"][l].rearrange("(kc p) n -> p kc n", p=128)
                    y2p = [[py2.tile([128, 512], F32, name=f"y2_{m}{n}")
                            for n in range(2)] for m in range(2)]
                    for kc in range(FF // 128):
                        f2t = st.tile([128, D], F32R, name="f2t")
                        nc.sync.dma_start(f2t[:], ff2v[:, kc, :])
                        for m in range(2):
                            for n in range(2):
                                nc.tensor.matmul(
                                    y2p[m][n][:],
                                    hT[:, kc, m * 128:(m + 1) * 128],
                                    f2t[:, n * 512:(n + 1) * 512],
                                    start=(kc == 0),
                                    stop=(kc == FF // 128 - 1))
                    for m in range(2):
                        for n in range(2):
                            nsl = slice(n * 512, (n + 1) * 512)
                            nc.vector.tensor_add(z[m][:, nsl], y2p[m][n][:],
                                                 xln[m][:, nsl])
                        nc.vector.tensor_add(z[m][:], z[m][:], ff2b_t[:])
                        layer_norm(x_own[m], z[m], ln2w_t, ln2b_t, sm)

                # ---- next xT AllGather ----
                with tc.tile_pool(name="ph_tx", bufs=1) as pp:
                    transpose_to_xt(x_own, t["xt_in"][l + 1], pp)
                nc.gpsimd.collective_compute(
                    "AllGather", mybir.AluOpType.bypass, replica_groups=rg,
                    ins=[t["xt_in"][l + 1][:, :]],
                    outs=[t["xt_out"][l + 1][:, :]])

        if debug_x:
            for m in range(2):
                nc.sync.dma_start(
                    t["dbg_x"][m * 128:(m + 1) * 128, :], x_own[m][:])

        # ---------------- vocab projection ----------------
        with tc.tile_pool(name="ph_voc", bufs=1) as pp, \
             tc.tile_pool(name="voc_st", bufs=3) as st, \
             tc.tile_pool(name="voc_sm", bufs=3) as sm, \
             tc.tile_pool(name="ps_voc", bufs=2, space="PSUM") as pv:
            XT = pp.tile([128, KC, NC, 256], F32R, name="XTf")
            for rb in range(NC):
                nc.gpsimd.dma_start(
                    XT[:, :, rb, :],
                    t["xt_out"][n_layers][rb * D:(rb + 1) * D, :]
                    .rearrange("(kc p) s -> p kc s", p=128))
            outwv = t["outw"].rearrange("(kc p) v -> p kc v", p=128)
            voff = 0
            for vc, vlen in enumerate(VCH):
                wv_t = st.tile([128, KC, 512], F32R, name="wvoc")
                nc.sync.dma_start(wv_t[:, :, 0:vlen],
                                  outwv[:, :, voff:voff + vlen])
                ob_t = sm.tile([128, 512], F32, name="ob")
                bcast_load(ob_t[:, 0:vlen], t["outb"][voff:voff + vlen])
                for sc in range(16):
                    rb, half = sc // 2, sc % 2
                    lpp = pv.tile([128, 512], F32, name="log_ps")
                    for kc in range(KC):
                        nc.tensor.matmul(
                            lpp[:, 0:vlen],
                            XT[:, kc, rb, half * 128:(half + 1) * 128],
                            wv_t[:, kc, 0:vlen],
                            start=(kc == 0), stop=(kc == KC - 1))
                    lo = sm.tile([128, 512], F32, name="log_sb")
                    nc.vector.tensor_add(lo[:, 0:vlen], lpp[:, 0:vlen],
                                         ob_t[:, 0:vlen])
                    nc.sync.dma_start(
                        t["logits"][sc * 128:(sc + 1) * 128,
                                    voff:voff + vlen],
                        lo[:, 0:vlen])
                voff += vlen


def _prepare_in_maps(inputs):
    ids = np.asarray(inputs["input_ids"]).reshape(S).astype(np.int32)
    cos, sin = _np_rope_tables()          # [S, DK]
    cosT = np.ascontiguousarray(np.concatenate([cos.T, cos.T], 0))  # [128, S]
    sinT = np.ascontiguousarray(np.concatenate([sin.T, sin.T], 0))
    masks = _diag_masks()
    f = np.float32
    in_maps = []
    for r in range(NC):
        hsl = slice(r * DHL, (r + 1) * DHL)
        im = {
            "ids": np.ascontiguousarray(
                ids[r * SL:(r + 1) * SL].reshape(2, 128, 1)),
            "token_emb": np.asarray(inputs["token_emb"], f),
            "pos_emb": np.ascontiguousarray(
                np.asarray(inputs["pos_emb"], f)[r * SL:(r + 1) * SL]),
            "wq": np.ascontiguousarray(np.asarray(inputs["wq"], f)[:, :, hsl]),
            "wk": np.ascontiguousarray(np.asarray(inputs["wk"], f)[:, :, hsl]),
            "wv": np.ascontiguousarray(np.asarray(inputs["wv"], f)[:, :, hsl]),
            "wo_w": np.asarray(inputs["wo_w"], f),
            "wo_b": np.asarray(inputs["wo_b"], f),
            "ln1_w": np.asarray(inputs["ln1_w"], f),
            "ln1_b": np.asarray(inputs["ln1_b"], f),
            "ln2_w": np.asarray(inputs["ln2_w"], f),
            "ln2_b": np.asarray(inputs["ln2_b"], f),
            "ff1_w": np.asarray(inputs["ff1_w"], f),
            "ff1_b": np.asarray(inputs["ff1_b"], f),
            "ff2_w": np.asarray(inputs["ff2_w"], f),
            "ff2_b": np.asarray(inputs["ff2_b"], f),
            "out_w": np.ascontiguousarray(
                np.asarray(inputs["out_w"], f)[:, r * VL:(r + 1) * VL]),
            "out_b": np.ascontiguousarray(
                np.asarray(inputs["out_b"], f)[r * VL:(r + 1) * VL]),
            "cosT": cosT,
            "sinT": sinT,
            "dmask": masks,
        }
        in_maps.append(im)
    return in_maps


def run(inputs, n_layers=L, debug_x=False, trace=False, tmpdir=None):
    key = (n_layers, debug_x)
    if key not in _CACHE:
        _CACHE[key] = build_program(n_layers, debug_x)
    nc = _CACHE[key]
    in_maps = _prepare_in_maps(inputs)
    res = run_bass_kernel_spmd(nc, in_maps, list(range(NC)), trace=trace,
                               tmpdir=tmpdir)
    return res


def kernel(**inputs):
    res = run(inputs)
    logits = np.concatenate([res.results[r]["logits"] for r in range(NC)],
                            axis=1)
    return logits.reshape(B, S, V)



# revision 9
# speedup vs baseline: 1.3670x; 1.3670x over previous
"""Trainium2 Bass kernel for a 4-layer dense transformer (B=1, S=2048, D=1024,
H=16, DK=64, FF=4096, V=50000) distributed over 8 NeuronCores.

Sharding:
  - Attention: tensor-parallel over heads (2 heads/core), full sequence.
  - LayerNorm / FFN / residual: sequence-parallel (256 rows/core), full width.
  - Vocab projection: sharded over vocab (6250 cols/core).
  - Glue per layer: AllGather of x^T (for QKV inputs) and AllToAll of the
    normalized ctx^T (delivers every head's dims for the core's own rows).
    One final AllGather before the vocab matmul.

All matmuls run in bf16 with fp32 PSUM accumulation; the residual stream is
kept in bf16. RoPE's rotate-half runs on the PE as a matmul with a constant
block-permutation matrix. Softmax has no max-subtraction (scores are O(1));
the causal mask is applied multiplicatively after exp and the denominator
comes from an appended ones-column in the PV matmul.
"""
import sys

if "/opt/trn_rl_repo" not in sys.path:
    sys.path.insert(0, "/opt/trn_rl_repo")

import contextlib

import ml_dtypes
import numpy as np

import concourse.bass as bass
import concourse.tile as tile
from concourse import bacc, mybir
from concourse.bass_utils import run_bass_kernel_spmd
from concourse.masks import make_identity

F32 = mybir.dt.float32
BF16 = mybir.dt.bfloat16
I32 = mybir.dt.int32
AF = mybir.ActivationFunctionType
ALU = mybir.AluOpType

NC = 8                    # cores
B, S, D, H, DK, FF, V, L = 1, 2048, 1024, 16, 64, 4096, 50000, 4
EPS = 1e-5
SCALE = 1.0 / np.sqrt(DK)
HL = H // NC              # heads per core = 2
DHL = HL * DK             # local head dims = 128
SL = S // NC              # rows per core = 256
VL = V // NC              # vocab per core = 6250
KC = D // 128             # contraction chunks over D = 8
VCH = [512] * 12 + [106]  # vocab free chunks (12*512+106 = 6250)
NBF = ml_dtypes.bfloat16

_CACHE = {}


def _np_rope_tables():
    inv_freq = 1.0 / (10000.0 ** (np.arange(0, DK, 2, dtype=np.float32) / DK))
    t = np.arange(S, dtype=np.float32)
    freqs = np.outer(t, inv_freq)                 # [S, DK/2]
    emb = np.concatenate([freqs, freqs], -1)      # [S, DK]
    return np.cos(emb), np.sin(emb)


def _diag_masks():
    # expT tile layout: [128 keys, 512 q]; for diag chunk d (0..3):
    # allowed iff q >= d*128 + k
    masks = np.zeros((4, 128, 512), np.float32)
    k = np.arange(128)[:, None]
    q = np.arange(512)[None, :]
    for d in range(4):
        masks[d] = (q >= d * 128 + k).astype(np.float32)
    return masks


def _rot_matrix():
    # rotT = rotm.T @ qT where qT is [128 (2 heads x 64 dims), s].
    # rot(q)[d] = -q[d+32] for d<32, +q[d-32] for 32<=d<64 (per head block).
    M = np.zeros((128, 128), np.float32)
    for o in (0, 64):
        for d in range(32):
            M[o + d, o + d + 32] = -1.0
            M[o + d + 32, o + d] = 1.0
    return np.ascontiguousarray(M.T)  # lhsT layout [j, d]


def build_program(n_layers=L):
    nc = bacc.Bacc("TRN2", target_bir_lowering=False, debug=False,
                   num_devices=NC)

    t = {}
    t["ids"] = nc.dram_tensor("ids", [2, 128, 1], I32, kind="ExternalInput")
    t["temb"] = nc.dram_tensor("token_emb", [V, D], F32, kind="ExternalInput")
    t["pemb"] = nc.dram_tensor("pos_emb", [SL, D], F32, kind="ExternalInput")
    t["wq"] = nc.dram_tensor("wq", [L, D, DHL], BF16, kind="ExternalInput")
    t["wk"] = nc.dram_tensor("wk", [L, D, DHL], BF16, kind="ExternalInput")
    t["wv"] = nc.dram_tensor("wv", [L, D, DHL], BF16, kind="ExternalInput")
    t["wo"] = nc.dram_tensor("wo_w", [L, D, D], BF16, kind="ExternalInput")
    t["wob"] = nc.dram_tensor("wo_b", [L, D], BF16, kind="ExternalInput")
    t["ln1w"] = nc.dram_tensor("ln1_w", [L, D], BF16, kind="ExternalInput")
    t["ln1b"] = nc.dram_tensor("ln1_b", [L, D], BF16, kind="ExternalInput")
    t["ln2w"] = nc.dram_tensor("ln2_w", [L, D], BF16, kind="ExternalInput")
    t["ln2b"] = nc.dram_tensor("ln2_b", [L, D], BF16, kind="ExternalInput")
    t["ff1"] = nc.dram_tensor("ff1_w", [L, D, FF], BF16, kind="ExternalInput")
    t["ff1b"] = nc.dram_tensor("ff1_b", [L, FF], F32, kind="ExternalInput")
    t["ff2"] = nc.dram_tensor("ff2_w", [L, FF, D], BF16, kind="ExternalInput")
    t["ff2b"] = nc.dram_tensor("ff2_b", [L, D], BF16, kind="ExternalInput")
    t["outw"] = nc.dram_tensor("out_w", [D, VL], BF16, kind="ExternalInput")
    t["cos"] = nc.dram_tensor("cosT", [128, S], BF16, kind="ExternalInput")
    t["sin"] = nc.dram_tensor("sinT", [128, S], BF16, kind="ExternalInput")
    t["rotm"] = nc.dram_tensor("rotm", [128, 128], BF16,
                               kind="ExternalInput")
    t["dmask"] = nc.dram_tensor("dmask", [4, 128, 512], BF16,
                                kind="ExternalInput")

    t["logits"] = nc.dram_tensor("logits", [S, VL], BF16,
                                 kind="ExternalOutput")

    # collective bounce buffers
    t["xt_in"] = [nc.dram_tensor(f"xt_in_{l}", [D, SL], BF16)
                  for l in range(n_layers + 1)]
    t["xt_out"] = [nc.dram_tensor(f"xt_out_{l}", [NC * D, SL], BF16,
                                  addr_space="Shared")
                   for l in range(n_layers + 1)]
    t["cx_in"] = [nc.dram_tensor(f"cx_in_{l}", [NC * 128, SL], BF16)
                  for l in range(n_layers)]
    t["cx_out"] = [nc.dram_tensor(f"cx_out_{l}", [NC * 128, SL], BF16)
                   for l in range(n_layers)]

    with tile.TileContext(nc) as tc:
        _build(nc, tc, t, n_layers)
    nc.compile()
    return nc


def _build(nc, tc, t, n_layers):
    rg = [list(range(NC))]
    es = contextlib.ExitStack()
    with es:
        const = es.enter_context(tc.tile_pool(name="const", bufs=1))
        glob = es.enter_context(tc.tile_pool(name="glob", bufs=1))
        wqkv_p = es.enter_context(tc.tile_pool(name="wqkv", bufs=2))
        wo_p = es.enter_context(tc.tile_pool(name="wop", bufs=1))
        st1 = es.enter_context(tc.tile_pool(name="st1", bufs=3))
        st2 = es.enter_context(tc.tile_pool(name="st2", bufs=3))
        bp = es.enter_context(tc.tile_pool(name="bp", bufs=2))
        stp = es.enter_context(tc.tile_pool(name="stp", bufs=4))

        # ---------------- constants ----------------
        ident = const.tile([128, 128], BF16)
        make_identity(nc, ident[:])
        cos_t = const.tile([128, S], BF16)
        sin_t = const.tile([128, S], BF16)
        nc.sync.dma_start(cos_t[:], t["cos"][:, :])
        nc.sync.dma_start(sin_t[:], t["sin"][:, :])
        rotm_t = const.tile([128, 128], BF16)
        nc.sync.dma_start(rotm_t[:], t["rotm"][:, :])
        dmask_t = const.tile([128, 4, 512], BF16)
        for d in range(4):
            nc.sync.dma_start(dmask_t[:, d, :], t["dmask"][d, :, :])
        eps_t = const.tile([128, 1], F32)
        nc.vector.memset(eps_t[:], EPS)

        def bcast_load(dst, src_1d):
            """DMA a [N] DRAM vector into a [P, N] tile, replicated."""
            p = dst.shape[0]
            ap = bass.AP(tensor=src_1d.tensor, offset=src_1d.offset,
                         ap=[[0, p]] + src_1d.ap)
            nc.sync.dma_start(dst, ap)

        # x_own[m]: [128, 1024] bf16, own rows (m=0: rows 0..127 of the
        # core's 256; m=1: rows 128..255)
        x_own = [glob.tile([128, D], BF16, name=f"x_own{m}")
                 for m in range(2)]

        def transpose_to_xt(src_tiles, dst_dram, pool, psp):
            """src [2][128, 1024] bf16 -> dst_dram [1024, 256] bf16 via PE."""
            xt_sb = pool.tile([128, KC, 256], BF16, name="xt_sb")
            for m in range(2):
                for kc in range(KC):
                    tp = psp.tile([128, 128], BF16, name="tp_ps")
                    nc.tensor.transpose(
                        tp[:], src_tiles[m][:, kc * 128:(kc + 1) * 128],
                        ident[:])
                    nc.vector.tensor_copy(
                        xt_sb[:, kc, m * 128:(m + 1) * 128], tp[:])
            for kc in range(KC):
                nc.sync.dma_start(
                    dst_dram[kc * 128:(kc + 1) * 128, :], xt_sb[:, kc, :])

        def layer_norm(dst, src, w_t, b_t):
            """dst (bf16) = LN(src f32) * w + b."""
            st = stp.tile([128, 2, 6], F32, name="bn_st")
            mv = stp.tile([128, 2], F32, name="bn_mv")
            for g in range(2):
                nc.vector.bn_stats(st[:, g, :],
                                   src[:, g * 512:(g + 1) * 512])
            nc.vector.bn_aggr(mv[:], st[:])
            rstd = stp.tile([128, 1], F32, name="rstd")
            nc.scalar.activation(rstd[:], mv[:, 1:2], AF.Sqrt,
                                 bias=eps_t[:])
            nc.vector.reciprocal(rstd[:], rstd[:])
            nc.vector.tensor_scalar(
                out=dst[:], in0=src[:], scalar1=mv[:, 0:1], scalar2=rstd[:],
                op0=ALU.subtract, op1=ALU.mult)
            nc.vector.tensor_mul(dst[:], dst[:], w_t[:])
            nc.vector.tensor_add(dst[:], dst[:], b_t[:])

        # ---------------- embedding ----------------
        with tc.tile_pool(name="emb", bufs=2) as emb, \
             tc.tile_pool(name="emb_ps", bufs=2, space="PSUM") as emb_ps:
            for m in range(2):
                idx_t = emb.tile([128, 1], I32, name="idx")
                nc.sync.dma_start(idx_t[:], t["ids"][m, :, :])
                gat = emb.tile([128, D], F32, name="gat")
                nc.gpsimd.indirect_dma_start(
                    out=gat[:], out_offset=None, in_=t["temb"][:, :],
                    in_offset=bass.IndirectOffsetOnAxis(ap=idx_t[:, :1],
                                                        axis=0))
                pos_t = emb.tile([128, D], F32, name="pos")
                nc.sync.dma_start(pos_t[:],
                                  t["pemb"][m * 128:(m + 1) * 128, :])
                nc.vector.tensor_add(x_own[m][:], gat[:], pos_t[:])
            transpose_to_xt(x_own, t["xt_in"][0], emb, emb_ps)
        nc.gpsimd.collective_compute(
            "AllGather", ALU.bypass, replica_groups=rg,
            ins=[t["xt_in"][0][:, :]], outs=[t["xt_out"][0][:, :]])

        # ---------------- layers ----------------
        for l in range(n_layers):
            les = contextlib.ExitStack()
            with les:
                lay = les.enter_context(
                    tc.tile_pool(name=f"lay{l}", bufs=1))
                pairp = les.enter_context(
                    tc.tile_pool(name=f"pair{l}", bufs=2))

                qTr = lay.tile([128, S], BF16, name="qTr")
                kTr = lay.tile([128, S], BF16, name="kTr")
                v_aug = [lay.tile([128, 16, 65], BF16, name=f"vaug{h}")
                         for h in range(HL)]
                ctxc = lay.tile([128, S], BF16, name="ctxc")
                xln = [lay.tile([128, D], BF16, name=f"xln{m}")
                       for m in range(2)]
                z = [lay.tile([128, D], F32, name=f"zz{m}")
                     for m in range(2)]

                for h in range(HL):
                    nc.vector.memset(v_aug[h][:, :, 64:65], 1.0)

                # ---- QKV + RoPE + attention (pipelined over chunks) ----
                wq_sb = wqkv_p.tile([128, KC, DHL], BF16, name="wq_sb")
                wk_sb = wqkv_p.tile([128, KC, DHL], BF16, name="wk_sb")
                wv_sb = wqkv_p.tile([128, KC, DHL], BF16, name="wv_sb")
                nc.sync.dma_start(
                    wq_sb[:],
                    t["wq"][l].rearrange("(kc p) m -> p kc m", p=128))
                nc.sync.dma_start(
                    wk_sb[:],
                    t["wk"][l].rearrange("(kc p) m -> p kc m", p=128))
                nc.sync.dma_start(
                    wv_sb[:],
                    t["wv"][l].rearrange("(kc p) m -> p kc m", p=128))

                with tc.tile_pool(name="pqk", bufs=2, space="PSUM") as pqk, \
                     tc.tile_pool(name="paux", bufs=1, space="PSUM") as paux, \
                     tc.tile_pool(name="psc", bufs=2, space="PSUM") as psc, \
                     tc.tile_pool(name="pct", bufs=1, space="PSUM") as pct, \
                     tc.tile_pool(name="att_e", bufs=4) as pe_, \
                     tc.tile_pool(name="att_sm", bufs=3) as asm:
                    # QKV per 512-col pair, RoPE fused
                    for p in range(4):
                        csl = slice(p * 512, (p + 1) * 512)
                        xt_c = pairp.tile([128, KC, 512], BF16, name="xt_c")
                        for rbo in range(2):
                            rb = p * 2 + rbo
                            nc.gpsimd.dma_start(
                                xt_c[:, :, rbo * 256:(rbo + 1) * 256],
                                t["xt_out"][l][rb * D:(rb + 1) * D, :]
                                .rearrange("(kc p) s -> p kc s", p=128))
                        for (w_sb, dstT) in ((wq_sb, qTr), (wk_sb, kTr)):
                            pt = pqk.tile([128, 512], F32, name="qk_ps")
                            for kc in range(KC):
                                nc.tensor.matmul(
                                    pt[:], w_sb[:, kc, :], xt_c[:, kc, :],
                                    start=(kc == 0), stop=(kc == KC - 1))
                            tc_sb = pairp.tile([128, 512], BF16,
                                               name="tc_sb")
                            nc.vector.tensor_copy(tc_sb[:], pt[:])
                            rot_ps = paux.tile([128, 512], F32,
                                               name="rot_ps")
                            nc.tensor.matmul(rot_ps[:], rotm_t[:],
                                             tc_sb[:], start=True,
                                             stop=True)
                            tmp = pairp.tile([128, 512], BF16,
                                             name="rope_tmp")
                            nc.vector.tensor_mul(tmp[:], tc_sb[:],
                                                 cos_t[:, csl])
                            rh = pairp.tile([128, 512], BF16,
                                            name="rope_rh")
                            nc.vector.tensor_mul(rh[:], rot_ps[:],
                                                 sin_t[:, csl])
                            nc.vector.tensor_add(dstT[:, csl], tmp[:],
                                                 rh[:])
                        vp = paux.tile([128, 512], F32, name="v_ps")
                        for half in range(4):
                            sc = p * 4 + half
                            hsl2 = slice(half * 128, (half + 1) * 128)
                            for kc in range(KC):
                                nc.tensor.matmul(
                                    vp[:, hsl2],
                                    xt_c[:, kc, hsl2],
                                    wv_sb[:, kc, :],
                                    start=(kc == 0), stop=(kc == KC - 1))
                            for h in range(HL):
                                nc.vector.tensor_copy(
                                    v_aug[h][:, sc, 0:64],
                                    vp[:, half * 128 + h * 64:
                                       half * 128 + (h + 1) * 64])

                    # attention
                    for qb in range(4):
                        nkc = (qb + 1) * 4
                        qsl = slice(qb * 512, (qb + 1) * 512)
                        ct_ps = [pct.tile([65, 512], F32, name=f"ct_ps{h}")
                                 for h in range(HL)]
                        for kc in range(nkc):
                            e_sb = [None, None]
                            for h in range(HL):
                                sc_ps = psc.tile([128, 512], F32,
                                                 name="sc_ps")
                                nc.tensor.matmul(
                                    sc_ps[:],
                                    kTr[h * 64:(h + 1) * 64,
                                        kc * 128:(kc + 1) * 128],
                                    qTr[h * 64:(h + 1) * 64, qsl],
                                    start=True, stop=True,
                                    tile_position=(64 * h, 0))
                                et = pe_.tile([128, 512], BF16, name="exp")
                                nc.scalar.activation(et[:], sc_ps[:],
                                                     AF.Exp,
                                                     scale=float(SCALE))
                                d = kc - qb * 4
                                if d >= 0:
                                    nc.vector.tensor_mul(
                                        et[:], et[:], dmask_t[:, d, :])
                                e_sb[h] = et
                            for h in range(HL):
                                nc.tensor.matmul(
                                    ct_ps[h][:], v_aug[h][:, kc, :],
                                    e_sb[h][:],
                                    start=(kc == 0), stop=(kc == nkc - 1))
                        for h in range(HL):
                            rec = asm.tile([1, 512], F32, name="rec")
                            nc.vector.tensor_copy(rec[:],
                                                  ct_ps[h][64:65, :])
                            bc = asm.tile([64, 512], F32, name="bc")
                            nc.gpsimd.partition_broadcast(bc[:], rec[:])
                            nc.vector.reciprocal(bc[:], bc[:])
                            nc.vector.tensor_mul(
                                ctxc[h * 64:(h + 1) * 64, qsl],
                                ct_ps[h][0:64, :], bc[:])

                # ---- AllToAll ctx ----
                for j in range(NC):
                    nc.sync.dma_start(
                        t["cx_in"][l][j * 128:(j + 1) * 128, :],
                        ctxc[:, j * SL:(j + 1) * SL])
                nc.gpsimd.collective_compute(
                    "AllToAll", ALU.bypass, replica_groups=rg,
                    ins=[t["cx_in"][l][:, :]], outs=[t["cx_out"][l][:, :]])

                # ---- out-proj + LN1 + FFN + LN2 ----
                ctxT = lay.tile([128, KC, 256], BF16, name="ctxT")
                for rb in range(NC):
                    nc.gpsimd.dma_start(
                        ctxT[:, rb, :],
                        t["cx_out"][l][rb * 128:(rb + 1) * 128, :])
                wo_sb = wo_p.tile([128, KC, D], BF16, name="wo_sb")
                nc.sync.dma_start(
                    wo_sb[:],
                    t["wo"][l].rearrange("(kc p) n -> p kc n", p=128))
                wob_t = bp.tile([128, D], BF16, name="wob")
                bcast_load(wob_t[:], t["wob"][l])
                ln1w_t = bp.tile([128, D], BF16, name="ln1w")
                ln1b_t = bp.tile([128, D], BF16, name="ln1b")
                bcast_load(ln1w_t[:], t["ln1w"][l])
                bcast_load(ln1b_t[:], t["ln1b"][l])

                with tc.tile_pool(name="pyo", bufs=2, space="PSUM") as pyo:
                    for m in range(2):
                        for n in range(2):
                            yp = pyo.tile([128, 512], F32, name="y_ps")
                            for kc in range(KC):
                                nc.tensor.matmul(
                                    yp[:],
                                    ctxT[:, kc, m * 128:(m + 1) * 128],
                                    wo_sb[:, kc, n * 512:(n + 1) * 512],
                                    start=(kc == 0), stop=(kc == KC - 1))
                            nsl = slice(n * 512, (n + 1) * 512)
                            nc.vector.tensor_add(z[m][:, nsl], yp[:],
                                                 x_own[m][:, nsl])
                        nc.vector.tensor_add(z[m][:], z[m][:], wob_t[:])
                        layer_norm(xln[m], z[m], ln1w_t, ln1b_t)

                # FFN
                with tc.tile_pool(name="ph1", bufs=2, space="PSUM") as ph1, \
                     tc.tile_pool(name="ptp", bufs=2, space="PSUM") as ptp, \
                     tc.tile_pool(name="py2", bufs=1, space="PSUM") as py2:
                    xlnT = lay.tile([128, KC, 256], BF16, name="xlnT")
                    for m in range(2):
                        for kc in range(KC):
                            tp = ptp.tile([128, 128], BF16, name="tp_ps")
                            nc.tensor.transpose(
                                tp[:], xln[m][:, kc * 128:(kc + 1) * 128],
                                ident[:])
                            nc.vector.tensor_copy(
                                xlnT[:, kc, m * 128:(m + 1) * 128], tp[:])
                    ff1b_t = stp.tile([128, FF // 128], F32, name="ff1b")
                    nc.sync.dma_start(
                        ff1b_t[:],
                        t["ff1b"][l].rearrange("(m p) -> p m", p=128))
                    hT = lay.tile([128, FF // 128, 256], BF16, name="hT")
                    ff1v = t["ff1"][l].rearrange("(kc p) m -> p kc m",
                                                 p=128)
                    for mh in range(FF // 128):
                        f1t = st1.tile([128, KC, 128], BF16, name="f1t")
                        nc.sync.dma_start(
                            f1t[:], ff1v[:, :, mh * 128:(mh + 1) * 128])
                        hp = ph1.tile([128, 256], F32, name="h_ps")
                        for kc in range(KC):
                            nc.tensor.matmul(
                                hp[:], f1t[:, kc, :], xlnT[:, kc, :],
                                start=(kc == 0), stop=(kc == KC - 1))
                        nc.scalar.activation(hT[:, mh, :], hp[:], AF.Gelu,
                                             bias=ff1b_t[:, mh:mh + 1])

                    ff2b_t = bp.tile([128, D], BF16, name="ff2b")
                    bcast_load(ff2b_t[:], t["ff2b"][l])
                    ln2w_t = bp.tile([128, D], BF16, name="ln2w")
                    ln2b_t = bp.tile([128, D], BF16, name="ln2b")
                    bcast_load(ln2w_t[:], t["ln2w"][l])
                    bcast_load(ln2b_t[:], t["ln2b"][l])
                    ff2v = t["ff2"][l].rearrange("(kc p) n -> p kc n",
                                                 p=128)
                    y2p = [[py2.tile([128, 512], F32, name=f"y2_{m}{n}")
                            for n in range(2)] for m in range(2)]
                    for kc in range(FF // 128):
                        f2t = st2.tile([128, D], BF16, name="f2t")
                        nc.sync.dma_start(f2t[:], ff2v[:, kc, :])
                        for m in range(2):
                            for n in range(2):
                                nc.tensor.matmul(
                                    y2p[m][n][:],
                                    hT[:, kc, m * 128:(m + 1) * 128],
                                    f2t[:, n * 512:(n + 1) * 512],
                                    start=(kc == 0),
                                    stop=(kc == FF // 128 - 1))
                    for m in range(2):
                        for n in range(2):
                            nsl = slice(n * 512, (n + 1) * 512)
                            nc.vector.tensor_add(z[m][:, nsl],
                                                 y2p[m][n][:],
                                                 xln[m][:, nsl])
                        nc.vector.tensor_add(z[m][:], z[m][:], ff2b_t[:])
                        layer_norm(x_own[m], z[m], ln2w_t, ln2b_t)

                # ---- next xT AllGather ----
                with tc.tile_pool(name="ph_tx", bufs=1) as pp, \
                     tc.tile_pool(name="ptx", bufs=2, space="PSUM") as ptx:
                    transpose_to_xt(x_own, t["xt_in"][l + 1], pp, ptx)
                nc.gpsimd.collective_compute(
                    "AllGather", ALU.bypass, replica_groups=rg,
                    ins=[t["xt_in"][l + 1][:, :]],
                    outs=[t["xt_out"][l + 1][:, :]])

        # ---------------- vocab projection ----------------
        with tc.tile_pool(name="ph_voc", bufs=1) as pp, \
             tc.tile_pool(name="voc_sm", bufs=3) as vsm, \
             tc.tile_pool(name="pvoc", bufs=2, space="PSUM") as pv:
            XT = pp.tile([128, KC, NC, 256], BF16, name="XTf")
            for rb in range(NC):
                nc.gpsimd.dma_start(
                    XT[:, :, rb, :],
                    t["xt_out"][n_layers][rb * D:(rb + 1) * D, :]
                    .rearrange("(kc p) s -> p kc s", p=128))
            outwv = t["outw"].rearrange("(kc p) v -> p kc v", p=128)
            voff = 0
            for vc, vlen in enumerate(VCH):
                wv_t = st2.tile([128, KC, 512], BF16, name="wvoc")
                nc.sync.dma_start(wv_t[:, :, 0:vlen],
                                  outwv[:, :, voff:voff + vlen])
                for sc in range(16):
                    rb, half = sc // 2, sc % 2
                    lpp = pv.tile([128, 512], F32, name="log_ps")
                    for kc in range(KC):
                        nc.tensor.matmul(
                            lpp[:, 0:vlen],
                            XT[:, kc, rb, half * 128:(half + 1) * 128],
                            wv_t[:, kc, 0:vlen],
                            start=(kc == 0), stop=(kc == KC - 1))
                    lo = vsm.tile([128, 512], BF16, name="log_sb")
                    nc.vector.tensor_copy(lo[:, 0:vlen], lpp[:, 0:vlen])
                    nc.sync.dma_start(
                        t["logits"][sc * 128:(sc + 1) * 128,
                                    voff:voff + vlen],
                        lo[:, 0:vlen])
                voff += vlen


def _prepare_in_maps(inputs):
    ids = np.asarray(inputs["input_ids"]).reshape(S).astype(np.int32)
    cos, sin = _np_rope_tables()          # [S, DK]
    cosT = np.ascontiguousarray(
        np.concatenate([cos.T, cos.T], 0)).astype(NBF)  # [128, S]
    sinT = np.ascontiguousarray(
        np.concatenate([sin.T, sin.T], 0)).astype(NBF)
    masks = _diag_masks().astype(NBF)
    rotm = _rot_matrix().astype(NBF)
    f = np.float32

    def b16(x):
        return np.ascontiguousarray(np.asarray(x, f)).astype(NBF)

    wq8 = b16(inputs["wq"])
    wk8 = b16(inputs["wk"])
    wv8 = b16(inputs["wv"])
    shared = {
        "token_emb": np.asarray(inputs["token_emb"], f),
        "wo_w": b16(inputs["wo_w"]),
        "wo_b": b16(inputs["wo_b"]),
        "ln1_w": b16(inputs["ln1_w"]),
        "ln1_b": b16(inputs["ln1_b"]),
        "ln2_w": b16(inputs["ln2_w"]),
        "ln2_b": b16(inputs["ln2_b"]),
        "ff1_w": b16(inputs["ff1_w"]),
        "ff1_b": np.asarray(inputs["ff1_b"], f),
        "ff2_w": b16(inputs["ff2_w"]),
        "ff2_b": b16(inputs["ff2_b"]),
        "cosT": cosT,
        "sinT": sinT,
        "rotm": rotm,
        "dmask": masks,
    }
    outw = np.asarray(inputs["out_w"], f)
    in_maps = []
    for r in range(NC):
        hsl = slice(r * DHL, (r + 1) * DHL)
        im = dict(shared)
        im.update({
            "ids": np.ascontiguousarray(
                ids[r * SL:(r + 1) * SL].reshape(2, 128, 1)),
            "pos_emb": np.ascontiguousarray(
                np.asarray(inputs["pos_emb"], f)[r * SL:(r + 1) * SL]),
            "wq": np.ascontiguousarray(wq8[:, :, hsl]),
            "wk": np.ascontiguousarray(wk8[:, :, hsl]),
            "wv": np.ascontiguousarray(wv8[:, :, hsl]),
            "out_w": np.ascontiguousarray(
                outw[:, r * VL:(r + 1) * VL]).astype(NBF),
        })
        in_maps.append(im)
    return in_maps


def run(inputs, n_layers=L, trace=False, tmpdir=None):
    key = n_layers
    if key not in _CACHE:
        _CACHE[key] = build_program(n_layers)
    nc = _CACHE[key]
    in_maps = _prepare_in_maps(inputs)
    res = run_bass_kernel_spmd(nc, in_maps, list(range(NC)), trace=trace,
                               tmpdir=tmpdir)
    return res


def kernel(**inputs):
    res = run(inputs)
    logits = np.concatenate(
        [np.asarray(res.results[r]["logits"], np.float32)
         for r in range(NC)], axis=1)
    logits += np.asarray(inputs["out_b"], np.float32)[None, :]
    return logits.reshape(B, S, V)
